# revision 22
# baseline (speedup 1.0000x reference)
"""AERGCN (2-layer R-GCN + bilinear attention pool) on 8 TRN2 NeuronCores.

Sharding: relation-expert. The 41 relations are padded to 48 slots; core c
owns slots [6c, 6c+6) (dummy slots get zero weights/adjacency and an exp-bias
of -1e4 so they vanish from the relation softmax). Each core reads only its
own slice of w_rgcn (the dominant HBM traffic), computes unnormalized
exp-weighted partial sums, and one AllReduce of [num | den] per layer
reconstitutes the softmax-combined hidden state. After layer 2 a
ReduceScatter hands batch c to core c, which runs the attention pool for
that batch alone.

Matmuls run in bf16 (f32 PSUM accumulate). Per-(slot,batch) pipeline:
  hidden = h @ [W_r | W_r @ score_w]          (accumulating matmuls)
  [logun | denom] = adj @ [hsw | ones]        (1 matmul, N=2, lhsT=adjT)
  e = exp(logun/denom + score_b); scr = e/denom
  num[b] += scr * (adj @ hidden)              (2 matmuls N=384 + DVE accum)
"""

import os
import sys

# The Bass NEFF executes through the axon PJRT backend; if the caller pinned
# jax to cpu before we ever import jax, lift the pin so axon devices resolve.
if "jax" not in sys.modules and os.environ.get("JAX_PLATFORMS") == "cpu":
    os.environ["JAX_PLATFORMS"] = ""

import numpy as np
import ml_dtypes

bf16 = ml_dtypes.bfloat16

B, S, F, R, NL = 8, 128, 768, 41, 2
NH, HD, EMB = 8, 96, 768
RLOC, NCORES, IC = 6, 8, 6
FE = F + 1  # 769: W with appended u column
EBIAS_MASK = -1e4

_CACHE = {}


def _build_graph():
    if "nc" in _CACHE:
        return _CACHE["nc"]

    import concourse.mybir as mybir
    import concourse.tile as tile
    from concourse import bacc
    from concourse.masks import make_identity

    dt = mybir.dt
    AF = mybir.ActivationFunctionType
    OP = mybir.AluOpType

    nc = bacc.Bacc("TRN2", target_bir_lowering=False, debug=False,
                   num_devices=NCORES)

    # ---------------- DRAM I/O (per-core shapes) ----------------
    xt = nc.dram_tensor("xt", [B, F, S], dt.bfloat16, kind="ExternalInput")
    xt6 = nc.dram_tensor("xt6", [F, S], dt.bfloat16, kind="ExternalInput")
    adjt = nc.dram_tensor("adjt", [RLOC, B, S, S], dt.bfloat16, kind="ExternalInput")
    w = nc.dram_tensor("w", [NL, RLOC, F, FE], dt.bfloat16, kind="ExternalInput")
    ebias = nc.dram_tensor("ebias", [NL, S, RLOC], dt.float32, kind="ExternalInput")
    wk_d = nc.dram_tensor("wk", [F, F], dt.bfloat16, kind="ExternalInput")
    wq_d = nc.dram_tensor("wq", [F, F], dt.bfloat16, kind="ExternalInput")
    wbil_d = nc.dram_tensor("wbil", [HD, HD], dt.bfloat16, kind="ExternalInput")
    wproj_d = nc.dram_tensor("wproj", [F, F], dt.bfloat16, kind="ExternalInput")
    bk_d = nc.dram_tensor("bk", [1, F], dt.bfloat16, kind="ExternalInput")
    bq_d = nc.dram_tensor("bq", [1, F], dt.bfloat16, kind="ExternalInput")
    bproj_d = nc.dram_tensor("bproj", [1, F], dt.bfloat16, kind="ExternalInput")
    qcol_d = nc.dram_tensor("qcol", [IC, S, 1], dt.bfloat16, kind="ExternalInput")
    out_d = nc.dram_tensor("out", [1, F], dt.float32, kind="ExternalOutput")

    groups = [list(range(NCORES))]

    with tile.TileContext(nc) as tc:
        with (
            tc.tile_pool(name="const", bufs=1) as constp,
            tc.tile_pool(name="wpool", bufs=12) as wpool,
            tc.tile_pool(name="hidp", bufs=16) as hidp,
            tc.tile_pool(name="adjp", bufs=16) as adjp,
            tc.tile_pool(name="hT", bufs=8) as hTp,
            tc.tile_pool(name="payl", bufs=8) as paylp,
            tc.tile_pool(name="tail", bufs=3) as tailp,
            tc.tile_pool(name="misc", bufs=2) as miscp,
            tc.tile_pool(name="dram", bufs=1, space="DRAM") as dramp,
            tc.tile_pool(name="ps_hid", bufs=2, space="PSUM") as ps_hid,
            tc.tile_pool(name="ps_ld", bufs=2, space="PSUM") as ps_ld,
            tc.tile_pool(name="ps_intm", bufs=2, space="PSUM") as ps_intm,
        ):
            ident_b = constp.tile([128, 128], dt.bfloat16, name="ident_b")
            make_identity(nc, ident_b)
            ident_f = constp.tile([128, 128], dt.float32, name="ident_f")
            make_identity(nc, ident_f)
            ones_row = constp.tile([1, 128], dt.bfloat16, name="ones_row")
            nc.vector.memset(ones_row, 1.0)
            one_sb = constp.tile([1, 1], dt.bfloat16, name="one_sb")
            nc.vector.memset(one_sb, 1.0)
            ebias_sb = constp.tile([S, NL * RLOC], dt.float32, name="ebias_sb")
            for l in range(NL):
                nc.sync.dma_start(ebias_sb[:, l * RLOC:(l + 1) * RLOC], ebias[l])

            # collective bounce buffers (DRAM pool so Tile tracks deps)
            GSZ = B // 2
            RF = RLOC - 1  # 5 full relation slots per core; slot 5 is shared r40
            ar1_full = dramp.tile([B, S, FE], dt.bfloat16, name="ar1_full")
            ar1_in = [ar1_full[g * GSZ:(g + 1) * GSZ] for g in range(2)]
            rs1x_out = dramp.tile([S, FE], dt.bfloat16, name="rs1x_out")
            ag_in = [dramp.tile([S, FE], dt.bfloat16, name=f"ag_in{l}")
                     for l in range(NL)]
            ag_out = [dramp.tile([B, S, FE], dt.bfloat16, name=f"ag_out{l}",
                                 addr_space="Shared") for l in range(NL)]
            ar1_out = [dramp.tile([GSZ, S, FE], dt.bfloat16, name=f"ar1o{g}",
                                  addr_space="Shared") for g in range(2)]
            rs_in = dramp.tile([B, S, FE], dt.bfloat16, name="rs_in")
            rs_out = dramp.tile([S, FE], dt.bfloat16, name="rs_out")
            warm_in = dramp.tile([8, 16], dt.bfloat16, name="warm_in")
            warm_ar = dramp.tile([8, 16], dt.bfloat16, name="warm_ar",
                                 addr_space="Shared")
            warm_rs = dramp.tile([1, 16], dt.bfloat16, name="warm_rs")

            # layer-1 lhsT: x^T per batch, [128(i within chunk), 6*128(s)]
            hT = []

            def load_hT(bb):
                t = hTp.tile([128, IC * S], dt.bfloat16, name=f"hT{bb}", tag="hT")
                nc.sync.dma_start(
                    t[:].rearrange("p (c s) -> p c s", c=IC),
                    xt[bb].rearrange("(c p) s -> p c s", p=128),
                )
                hT.append(t)

            payload = [
                [paylp.tile([S, FE], dt.float32, name=f"pay{l}_{bb}", tag=f"pay{l}")
                 for bb in range(B)]
                for l in range(NL)
            ]

            def rgcn_group(l, g, hT_tiles, carry=None):
                bs = list(range(g * GSZ, (g + 1) * GSZ))
                denacc = tailp.tile([S, GSZ], dt.float32, name=f"den{l}{g}",
                                    tag="denacc")

                def phase_b(ll, rr, bi, bb, adjT, hid, scr):
                    for half in range(2):
                        c0 = half * 384
                        intm = ps_intm.tile([S, 384], dt.float32,
                                            name=f"in{ll}{g}{rr}{bb}{half}",
                                            tag="intm")
                        nc.tensor.matmul(
                            intm[:], lhsT=adjT[:],
                            rhs=hid[:, c0:c0 + 384],
                            start=True, stop=True)
                        dst = payload[ll][bb][:, c0:c0 + 384]
                        if rr == 0:
                            nc.vector.tensor_scalar(
                                dst, intm[:], scr[:, bi:bi + 1], None,
                                OP.mult)
                        else:
                            nc.vector.scalar_tensor_tensor(
                                dst, intm[:], scr[:, bi:bi + 1], dst,
                                OP.mult, OP.add)

                def ship(ll, gg, bi, bb, den):
                    nc.vector.tensor_copy(payload[ll][bb][:, F:FE],
                                          den[:, bi:bi + 1])
                    agsb = miscp.tile([S, FE], dt.bfloat16, name=f"ag{ll}{bb}",
                                      tag="agsb", bufs=4)
                    nc.sync.dma_start(agsb[:], ag_out[ll][bb])
                    nc.vector.scalar_tensor_tensor(
                        payload[ll][bb][:], agsb[:], 0.125,
                        payload[ll][bb][:], OP.mult, OP.add)
                    pyc = miscp.tile([S, FE], dt.bfloat16, name=f"pyc{ll}{bb}",
                                     tag="pyc")
                    nc.scalar.copy(pyc[:], payload[ll][bb][:])
                    if ll == 0:
                        nc.sync.dma_start(ar1_in[gg][bi], pyc[:])
                    else:
                        nc.sync.dma_start(rs_in[bb], pyc[:])

                prev = None
                for r in range(RF):
                    wt = []
                    for ic in range(IC):
                        t = wpool.tile([128, FE], dt.bfloat16,
                                       name=f"w{l}_{r}_{g}_{ic}", tag="wt")
                        nc.sync.dma_start(t[:], w[l, r, ic * 128:(ic + 1) * 128, :])
                        wt.append(t)
                    ld_ps = ps_ld.tile([128, GSZ, 4], dt.float32,
                                       name=f"ld{l}{g}{r}", tag="ld")
                    adjT_l, hid_l = [], []
                    for bi, bb in enumerate(bs):
                        adjT = adjp.tile([S, S], dt.bfloat16,
                                         name=f"adjT{l}{g}_{r}_{bb}", tag="adjT")
                        nc.sync.dma_start(adjT[:], adjt[r, bb])
                        hid_ps = ps_hid.tile([S, FE], dt.float32,
                                             name=f"hps{l}{g}_{r}_{bb}",
                                             tag="hid")
                        for c0, c1 in ((0, 512), (512, FE)):
                            for ic in range(IC):
                                lhsT = hT_tiles[bb][:, ic * S:(ic + 1) * S]
                                nc.tensor.matmul(
                                    hid_ps[:, c0:c1], lhsT=lhsT,
                                    rhs=wt[ic][:, c0:c1],
                                    start=(ic == 0), stop=(ic == IC - 1))
                        hid = hidp.tile([S, FE + 1], dt.bfloat16,
                                        name=f"hid{l}{g}_{r}_{bb}", tag="hid")
                        nc.scalar.copy(hid[:, :FE], hid_ps[:])
                        nc.gpsimd.memset(hid[:, FE:FE + 1], 1.0)
                        nc.tensor.matmul(
                            ld_ps[:, bi, 0:2],
                            lhsT=adjT[:],
                            rhs=hid[:, F:FE + 1],
                            start=True, stop=True,
                        )
                        adjT_l.append(adjT)
                        hid_l.append(hid)
                        if prev is not None:
                            phase_b(l, r - 1, bi, bb, prev[0][bi], prev[1][bi],
                                    prev[2])
                        elif carry is not None:
                            cl, cg, cprev, cden = carry
                            phase_b(cl, RLOC - 1, bi, cg * GSZ + bi,
                                    cprev[0][bi], cprev[1][bi], cprev[2])
                        if carry is not None and r == 2:
                            cl, cg, cprev, cden = carry
                            ship(cl, cg, bi, cg * GSZ + bi, cden)
                            if cl == 0 and bi == GSZ - 1:
                                nc.gpsimd.collective_compute(
                                    "AllReduce", OP.add, replica_groups=groups,
                                    ins=[ar1_in[cg].opt()],
                                    outs=[ar1_out[cg].opt()],
                                )
                    # ---- tail for slot r (batched over the group) ----
                    dsafe = tailp.tile([S, GSZ], dt.float32, name=f"ds{l}{g}{r}",
                                       tag="ds")
                    nc.vector.tensor_scalar_max(dsafe[:], ld_ps[:, :, 1], 1e-30)
                    rec = tailp.tile([S, GSZ], dt.float32, name=f"rc{l}{g}{r}",
                                     tag="rc")
                    nc.vector.reciprocal(rec[:], dsafe[:])
                    tmul = tailp.tile([S, GSZ], dt.float32, name=f"tm{l}{g}{r}",
                                      tag="tm")
                    nc.vector.tensor_mul(tmul[:], ld_ps[:, :, 0], rec[:])
                    ee = tailp.tile([S, GSZ], dt.float32, name=f"ee{l}{g}{r}",
                                    tag="ee")
                    nc.scalar.activation(ee[:], tmul[:], AF.Exp,
                                         bias=ebias_sb[:, l * RLOC + r:
                                                       l * RLOC + r + 1])
                    scr = tailp.tile([S, GSZ], dt.float32, name=f"sc{l}{g}{r}",
                                     tag="sc")
                    nc.vector.tensor_mul(scr[:], ee[:], rec[:])
                    if r == 0:
                        nc.vector.tensor_copy(denacc[:], ee[:])
                    else:
                        nc.vector.tensor_add(denacc[:], denacc[:], ee[:])
                    prev = (adjT_l, hid_l, scr)
                return (l, g, prev, denacc)

            def drain(carry):
                cl, cg, cprev, cden = carry
                for bi in range(GSZ):
                    bb = cg * GSZ + bi
                    for half in range(2):
                        c0 = half * 384
                        intm = ps_intm.tile([S, 384], dt.float32,
                                            name=f"dr{cl}{cg}{bb}{half}",
                                            tag="intm")
                        nc.tensor.matmul(
                            intm[:], lhsT=cprev[0][bi][:],
                            rhs=cprev[1][bi][:, c0:c0 + 384],
                            start=True, stop=True)
                        dst = payload[cl][bb][:, c0:c0 + 384]
                        nc.vector.scalar_tensor_tensor(
                            dst, intm[:], cprev[2][:, bi:bi + 1], dst,
                            OP.mult, OP.add)
                    nc.vector.tensor_copy(payload[cl][bb][:, F:FE],
                                          cden[:, bi:bi + 1])
                    agsb = miscp.tile([S, FE], dt.bfloat16, name=f"ag{cl}{bb}",
                                      tag="agsb", bufs=4)
                    nc.sync.dma_start(agsb[:], ag_out[cl][bb])
                    nc.vector.scalar_tensor_tensor(
                        payload[cl][bb][:], agsb[:], 0.125,
                        payload[cl][bb][:], OP.mult, OP.add)
                    pyc = miscp.tile([S, FE], dt.bfloat16, name=f"pyc{cl}{bb}",
                                     tag="pyc")
                    nc.scalar.copy(pyc[:], payload[cl][bb][:])
                    if cl == 0:
                        nc.sync.dma_start(ar1_in[cg][bi], pyc[:])
                    else:
                        nc.sync.dma_start(rs_in[bb], pyc[:])
                if cl == 0:
                    nc.gpsimd.collective_compute(
                        "AllReduce", OP.add, replica_groups=groups,
                        ins=[ar1_in[cg].opt()], outs=[ar1_out[cg].opt()],
                    )

            def h2_prep(g, h2T):
                for bi, bb in enumerate(range(g * GSZ, (g + 1) * GSZ)):
                    raw = miscp.tile([S, FE], dt.bfloat16, name=f"raw{bb}",
                                     tag="raw")
                    nc.sync.dma_start(raw[:], ar1_out[g][bi])
                    rd = miscp.tile([S, 1], dt.float32, name=f"rd{bb}", tag="rd")
                    nc.vector.reciprocal(rd[:], raw[:, F:FE])
                    h2 = miscp.tile([S, F], dt.bfloat16, name=f"h2_{bb}",
                                    tag="h2")
                    nc.scalar.activation(h2[:], raw[:, :F], AF.Relu, scale=rd[:])
                    t = hTp.tile([128, IC * S], dt.bfloat16, name=f"h2T{bb}",
                                 tag="hT")
                    for ic in range(IC):
                        tp = ps_ld.tile([128, 128], dt.bfloat16,
                                        name=f"tp{bb}_{ic}", tag="ld")
                        nc.tensor.transpose(tp[:], h2[:, ic * 128:(ic + 1) * 128],
                                            ident_b[:])
                        nc.scalar.copy(t[:, ic * S:(ic + 1) * S], tp[:])
                    h2T.append(t)

            def slot5(l, hT6):
                # shared relation 40: this core computes its own batch only;
                # the AllGather's slice index doubles as the batch index
                wt6 = []
                for ic in range(IC):
                    t = wpool.tile([128, FE], dt.bfloat16, name=f"w6_{l}_{ic}",
                                   tag="wt")
                    nc.sync.dma_start(t[:], w[l, RF, ic * 128:(ic + 1) * 128, :])
                    wt6.append(t)
                adjT6 = adjp.tile([S, S], dt.bfloat16, name=f"adjT6_{l}",
                                  tag="adjT")
                nc.sync.dma_start(adjT6[:], adjt[RF, 0])
                hid_ps = ps_hid.tile([S, FE], dt.float32, name=f"hps6{l}",
                                     tag="hid")
                for c0, c1 in ((0, 512), (512, FE)):
                    for ic in range(IC):
                        nc.tensor.matmul(hid_ps[:, c0:c1],
                                         lhsT=hT6[:, ic * S:(ic + 1) * S],
                                         rhs=wt6[ic][:, c0:c1],
                                         start=(ic == 0), stop=(ic == IC - 1))
                hid6 = hidp.tile([S, FE + 1], dt.bfloat16, name=f"hid6{l}",
                                 tag="hid")
                nc.scalar.copy(hid6[:, :FE], hid_ps[:])
                nc.gpsimd.memset(hid6[:, FE:FE + 1], 1.0)
                ld6 = ps_ld.tile([128, 4], dt.float32, name=f"ld6{l}", tag="ld")
                nc.tensor.matmul(ld6[:, 0:2], lhsT=adjT6[:],
                                 rhs=hid6[:, F:FE + 1], start=True, stop=True)
                ds6 = tailp.tile([S, 1], dt.float32, name=f"ds6{l}", tag="ds")
                nc.vector.tensor_scalar_max(ds6[:], ld6[:, 1:2], 1e-30)
                rc6 = tailp.tile([S, 1], dt.float32, name=f"rc6{l}", tag="rc")
                nc.vector.reciprocal(rc6[:], ds6[:])
                tm6 = tailp.tile([S, 1], dt.float32, name=f"tm6{l}", tag="tm")
                nc.vector.tensor_mul(tm6[:], ld6[:, 0:1], rc6[:])
                ee6 = tailp.tile([S, 1], dt.float32, name=f"ee6{l}", tag="ee")
                nc.scalar.activation(ee6[:], tm6[:], AF.Exp,
                                     bias=ebias_sb[:, l * RLOC + RF:
                                                   l * RLOC + RF + 1])
                sc6 = tailp.tile([S, 1], dt.float32, name=f"sc6{l}", tag="sc")
                nc.vector.tensor_mul(sc6[:], ee6[:], rc6[:])
                pay6 = miscp.tile([S, FE], dt.float32, name=f"pay6{l}",
                                  tag="pay6")
                for half in range(2):
                    c0 = half * 384
                    intm = ps_intm.tile([S, 384], dt.float32,
                                        name=f"in6{l}{half}", tag="intm")
                    nc.tensor.matmul(intm[:], lhsT=adjT6[:],
                                     rhs=hid6[:, c0:c0 + 384],
                                     start=True, stop=True)
                    nc.vector.tensor_scalar(pay6[:, c0:c0 + 384], intm[:],
                                            sc6[:], None, OP.mult)
                nc.vector.tensor_copy(pay6[:, F:FE], ee6[:])
                pyc6 = miscp.tile([S, FE], dt.bfloat16, name=f"pyc6{l}",
                                  tag="pyc")
                nc.scalar.copy(pyc6[:], pay6[:])
                nc.sync.dma_start(ag_in[l][:], pyc6[:])
                nc.gpsimd.collective_compute(
                    "AllGather", OP.bypass, replica_groups=groups,
                    ins=[ag_in[l].opt()], outs=[ag_out[l].opt()])

            # ============ wavefront schedule ============
            hT6a = hTp.tile([128, IC * S], dt.bfloat16, name="hT6a", tag="hT")
            nc.sync.dma_start(
                hT6a[:].rearrange("p (c s) -> p c s", c=IC),
                xt6.rearrange("(c p) s -> p c s", p=128),
            )
            slot5(0, hT6a)
            for bb in range(GSZ):
                load_hT(bb)
            c00 = rgcn_group(0, 0, hT)
            for bb in range(GSZ, B):
                load_hT(bb)
            # --- attention q-path: depends only on q/wq/wbil; run first ---
            qc = []
            for ic in range(IC):
                t = constp.tile([S, 1], dt.bfloat16, name=f"qc{ic}")
                nc.sync.dma_start(t[:], qcol_d[ic])
                qc.append(t)
            bq_sb = constp.tile([1, F], dt.bfloat16, name="bq_sb")
            nc.sync.dma_start(bq_sb[:], bq_d[:])
            wbil_sb = constp.tile([HD, HD], dt.bfloat16, name="wbil_sb")
            nc.sync.dma_start(wbil_sb[:], wbil_d[:])
            one_f = constp.tile([1, 1], dt.bfloat16, name="one_f")
            nc.vector.memset(one_f, 1.0)
            qxT_ps = ps_intm.tile([HD, NH, 4], dt.float32, name="qxT_ps",
                                  tag="intm")
            wqts = []
            for ic in range(IC):
                wqt = wpool.tile([128, F], dt.bfloat16, name=f"wq{ic}",
                                 tag="wt")
                nc.sync.dma_start(wqt[:], wq_d[ic * 128:(ic + 1) * 128, :])
                wqts.append(wqt)
            for hh in range(NH):
                for ic in range(IC):
                    nc.tensor.matmul(qxT_ps[:, hh, 0:1],
                                     lhsT=wqts[ic][:, hh * HD:(hh + 1) * HD],
                                     rhs=qc[ic][:],
                                     start=(ic == 0), stop=False)
                nc.tensor.matmul(qxT_ps[:, hh, 0:1],
                                 lhsT=bq_sb[:, hh * HD:(hh + 1) * HD],
                                 rhs=one_f[:], start=False, stop=True)
            qxT = constp.tile([HD, NH], dt.bfloat16, name="qxT")
            nc.scalar.copy(qxT[:], qxT_ps[:, :, 0])
            qw_ps = ps_intm.tile([HD, NH, 4], dt.float32, name="qw_ps",
                                 tag="intm")
            for hh in range(NH):
                nc.tensor.matmul(qw_ps[:, hh, 0:1], lhsT=wbil_sb[:],
                                 rhs=qxT[:, hh:hh + 1], start=True,
                                 stop=True)
            qwT = constp.tile([HD, NH], dt.bfloat16, name="qwT")
            nc.scalar.copy(qwT[:], qw_ps[:, :, 0])

            c01 = rgcn_group(0, 1, hT, carry=c00)
            drain(c01)
            nc.gpsimd.collective_compute(
                "ReduceScatter", OP.add, replica_groups=groups,
                ins=[ar1_full.opt()], outs=[rs1x_out.opt()])
            h2T = []
            if True:
                h2_prep(0, h2T)
                c10 = rgcn_group(1, 0, h2T)
                # slot-5 of layer 2: batch c's reduced L1 state via rs1x
                raw6 = miscp.tile([S, FE], dt.bfloat16, name="raw6", tag="raw")
                nc.sync.dma_start(raw6[:], rs1x_out[:])
                rd6 = miscp.tile([S, 1], dt.float32, name="rd6", tag="rd")
                nc.vector.reciprocal(rd6[:], raw6[:, F:FE])
                h26 = miscp.tile([S, F], dt.bfloat16, name="h26", tag="h2")
                nc.scalar.activation(h26[:], raw6[:, :F], AF.Relu, scale=rd6[:])
                hT6b = hTp.tile([128, IC * S], dt.bfloat16, name="hT6b",
                                tag="hT")
                for ic in range(IC):
                    tp6 = ps_ld.tile([128, 128], dt.bfloat16, name=f"tp6{ic}",
                                     tag="ld")
                    nc.tensor.transpose(tp6[:], h26[:, ic * 128:(ic + 1) * 128],
                                        ident_b[:])
                    nc.scalar.copy(hT6b[:, ic * S:(ic + 1) * S], tp6[:])
                slot5(1, hT6b)
                h2_prep(1, h2T)
                c11 = rgcn_group(1, 1, h2T, carry=c10)
                drain(c11)
                nc.gpsimd.collective_compute(
                    "ReduceScatter", OP.add, replica_groups=groups,
                    ins=[rs_in.opt()], outs=[rs_out.opt()],
                )

                # =================== attention (batch = core id) ==========
                raw = miscp.tile([S, FE], dt.bfloat16, name="rawf", tag="raw")
                nc.sync.dma_start(raw[:], rs_out[:])
                rd = miscp.tile([S, 1], dt.float32, name="rdf", tag="rd")
                nc.vector.reciprocal(rd[:], raw[:, F:FE])
                hf = miscp.tile([S, F], dt.bfloat16, name="hf", tag="h2")
                nc.scalar.activation(hf[:], raw[:, :F], AF.Relu, scale=rd[:])
                hfT = hTp.tile([128, IC * S], dt.bfloat16, name="hfT", tag="hT")
                for ic in range(IC):
                    tp = ps_ld.tile([128, 128], dt.bfloat16, name=f"ftp{ic}",
                                    tag="ld")
                    nc.tensor.transpose(tp[:], hf[:, ic * 128:(ic + 1) * 128],
                                        ident_b[:])
                    nc.scalar.copy(hfT[:, ic * S:(ic + 1) * S], tp[:])

                # kx = hf @ wk + bk   [S, 768]
                kx_ps = ps_hid.tile([S, F], dt.float32, name="kx_ps", tag="hid")
                bk_sb = constp.tile([1, F], dt.bfloat16, name="bk_sb")
                nc.sync.dma_start(bk_sb[:], bk_d[:])
                wkts = []
                for ic in range(IC):
                    wkt = wpool.tile([128, F], dt.bfloat16, name=f"wk{ic}",
                                     tag="wt")
                    nc.sync.dma_start(wkt[:], wk_d[ic * 128:(ic + 1) * 128, :])
                    wkts.append(wkt)
                    lhsT = hfT[:, ic * S:(ic + 1) * S]
                    nc.tensor.matmul(kx_ps[:, 0:512], lhsT=lhsT,
                                     rhs=wkt[:, 0:512],
                                     start=(ic == 0), stop=False)
                    nc.tensor.matmul(kx_ps[:, 512:F], lhsT=lhsT,
                                     rhs=wkt[:, 512:F],
                                     start=(ic == 0), stop=False)
                nc.tensor.matmul(kx_ps[:, 0:512], lhsT=ones_row[:],
                                 rhs=bk_sb[:, 0:512], start=False, stop=True)
                nc.tensor.matmul(kx_ps[:, 512:F], lhsT=ones_row[:],
                                 rhs=bk_sb[:, 512:F], start=False, stop=True)
                kx = miscp.tile([S, F], dt.bfloat16, name="kx", tag="h2")
                nc.scalar.copy(kx[:], kx_ps[:])
                # kxT per head directly: kxT_h = sum_ic wk[ic,h]^T @ hfT[ic]
                kxT = miscp.tile([HD, NH * S], dt.bfloat16, name="kxT",
                                 tag="kxT")
                for hh in range(NH):
                    ktp = ps_intm.tile([HD, S], dt.float32, name=f"ktp{hh}",
                                       tag="intm")
                    for ic in range(IC):
                        nc.tensor.matmul(
                            ktp[:],
                            lhsT=wkts[ic][:, hh * HD:(hh + 1) * HD],
                            rhs=hfT[:, ic * S:(ic + 1) * S],
                            start=(ic == 0), stop=False)
                    nc.tensor.matmul(ktp[:],
                                     lhsT=bk_sb[:, hh * HD:(hh + 1) * HD],
                                     rhs=ones_row[:], start=False, stop=True)
                    nc.scalar.copy(kxT[:, hh * S:(hh + 1) * S], ktp[:])

                # scoreT[:,h] = kx_h @ qwT_h    [128, 8]
                sc_ps = ps_intm.tile([S, NH, 4], dt.float32, name="sc_ps",
                                     tag="intm")
                for hh in range(NH):
                    nc.tensor.matmul(sc_ps[:, hh, 0:1],
                                     lhsT=kxT[:, hh * S:(hh + 1) * S],
                                     rhs=qwT[:, hh:hh + 1], start=True,
                                     stop=True)
                sc_sb = miscp.tile([S, NH], dt.float32, name="sc_sb", tag="scb")
                nc.scalar.copy(sc_sb[:], sc_ps[:, :, 0])
                # score rows [8, 128]
                srow_ps = ps_ld.tile([NH, S], dt.float32, name="srow", tag="ld")
                nc.tensor.transpose(srow_ps[:], sc_sb[:], ident_f[:])
                negmax = miscp.tile([NH, 1], dt.float32, name="negmax", tag="sm")
                nc.vector.tensor_reduce(negmax[:], srow_ps[:],
                                        mybir.AxisListType.X, OP.max,
                                        negate=True)
                esc = miscp.tile([NH, S], dt.float32, name="esc", tag="esc")
                sumexp = miscp.tile([NH, 1], dt.float32, name="sumexp", tag="sm")
                nc.scalar.activation(esc[:], srow_ps[:], AF.Exp, bias=negmax[:],
                                     accum_out=sumexp[:])
                rsm = miscp.tile([NH, 1], dt.float32, name="rsm", tag="sm")
                nc.vector.reciprocal(rsm[:], sumexp[:])
                attn = miscp.tile([NH, S], dt.bfloat16, name="attn", tag="esc")
                nc.vector.tensor_scalar_mul(attn[:], esc[:], rsm[:])
                # attnT [128, 8]
                at_ps = ps_ld.tile([S, NH], dt.bfloat16, name="at_ps", tag="ld")
                nc.tensor.transpose(at_ps[:], attn[:], ident_b[:NH, :NH])
                attnT = miscp.tile([S, NH], dt.bfloat16, name="attnT", tag="scb")
                nc.scalar.copy(attnT[:], at_ps[:])
                # o[0, h*96:(h+1)*96] = attn_h @ kx_h  (bank-safe [1,8,128] tile)
                o_psA = ps_intm.tile([1, 4, 128], dt.float32, name="o_psA",
                                     tag="intm")
                o_psB = ps_intm.tile([1, 4, 128], dt.float32, name="o_psB",
                                     tag="intm")
                for hh in range(NH):
                    tgt = o_psA if hh < 4 else o_psB
                    nc.tensor.matmul(tgt[:, hh % 4, :HD],
                                     lhsT=attnT[:, hh:hh + 1],
                                     rhs=kx[:, hh * HD:(hh + 1) * HD],
                                     start=True, stop=True)
                o_sb = miscp.tile([1, F], dt.bfloat16, name="o_sb", tag="qx")
                nc.scalar.copy(o_sb[:, 0:384], o_psA[:, :, :HD])
                nc.scalar.copy(o_sb[:, 384:F], o_psB[:, :, :HD])
                # oT [128, 6]
                oT_ps = ps_ld.tile([S, IC, 4], dt.bfloat16, name="oT_ps",
                                    tag="ld")
                for ic in range(IC):
                    nc.tensor.transpose(oT_ps[:, ic, 0:1],
                                        o_sb[:, ic * 128:(ic + 1) * 128],
                                        ident_b[:1, :1])
                oT = miscp.tile([S, IC], dt.bfloat16, name="oT", tag="scb")
                nc.scalar.copy(oT[:], oT_ps[:, :, 0])
                # res = o @ wproj + bproj
                res_ps = ps_hid.tile([1, 512], dt.float32, name="res_ps",
                                     tag="hid")
                res_ps2 = ps_hid.tile([1, 256], dt.float32, name="res_ps2",
                                      tag="hid")
                bp_sb = constp.tile([1, F], dt.bfloat16, name="bp_sb")
                nc.sync.dma_start(bp_sb[:], bproj_d[:])
                for ic in range(IC):
                    wpt = wpool.tile([128, F], dt.bfloat16, name=f"wp{ic}",
                                     tag="wt")
                    nc.sync.dma_start(wpt[:], wproj_d[ic * 128:(ic + 1) * 128, :])
                    nc.tensor.matmul(res_ps[:], lhsT=oT[:, ic:ic + 1],
                                     rhs=wpt[:, 0:512],
                                     start=(ic == 0), stop=False)
                    nc.tensor.matmul(res_ps2[:], lhsT=oT[:, ic:ic + 1],
                                     rhs=wpt[:, 512:F],
                                     start=(ic == 0), stop=False)
                nc.tensor.matmul(res_ps[:], lhsT=one_sb[:], rhs=bp_sb[:, 0:512],
                                 start=False, stop=True)
                nc.tensor.matmul(res_ps2[:], lhsT=one_sb[:], rhs=bp_sb[:, 512:F],
                                 start=False, stop=True)
                res_sb = miscp.tile([1, F], dt.float32, name="res_sb", tag="res")
                nc.scalar.copy(res_sb[:, 0:512], res_ps[:])
                nc.scalar.copy(res_sb[:, 512:F], res_ps2[:])
                nc.sync.dma_start(out_d[:], res_sb[:])

    nc.compile()
    _CACHE["nc"] = nc
    return nc


def _prep_inputs(x, adj, q, w_rgcn, score_w, score_b, wk, bk, wq, bq, wbil,
                 wproj, bproj):
    f32 = np.float32
    x = np.asarray(x, f32)
    adj = np.asarray(adj, f32)
    q = np.asarray(q, f32)
    w_rgcn = np.asarray(w_rgcn, f32)
    score_w = np.asarray(score_w, f32)
    score_b = np.asarray(score_b, f32)

    u = np.einsum("lrio,lo->lri", w_rgcn, score_w).astype(f32)
    w_ext = np.concatenate([w_rgcn, u[..., None]], axis=-1)  # [2,41,768,769]

    xt_all = np.ascontiguousarray(x.transpose(0, 2, 1)).astype(bf16)  # [B,F,S]

    shared = {
        "xt": xt_all,
        "wk": np.asarray(wk, f32).astype(bf16),
        "wq": np.asarray(wq, f32).astype(bf16),
        "wbil": np.asarray(wbil, f32).astype(bf16),
        "wproj": np.asarray(wproj, f32).astype(bf16),
        "bk": np.asarray(bk, f32).reshape(1, F).astype(bf16),
        "bq": np.asarray(bq, f32).reshape(1, F).astype(bf16),
        "bproj": np.asarray(bproj, f32).reshape(1, F).astype(bf16),
    }

    RF = RLOC - 1  # 5 full relation slots; slot 5 = shared relation 40
    in_maps = []
    for c in range(NCORES):
        sl = slice(c * RF, c * RF + RF)  # relations 5c .. 5c+4 (all real)
        w_c = np.zeros((NL, RLOC, F, FE), f32)
        w_c[:, :RF] = w_ext[:, sl]
        w_c[:, RF] = w_ext[:, R - 1]
        adjt_c = np.zeros((RLOC, B, S, S), f32)
        adjt_c[:RF] = adj[:, sl].transpose(1, 0, 3, 2)
        adjt_c[RF, 0] = adj[c, R - 1].T
        eb_c = np.empty((NL, S, RLOC), f32)
        for l in range(NL):
            eb_c[l] = score_b[l]
        m = dict(shared)
        m["w"] = w_c.astype(bf16)
        m["adjt"] = np.ascontiguousarray(adjt_c).astype(bf16)
        m["ebias"] = eb_c
        m["qcol"] = q[c].reshape(IC, S, 1).astype(bf16)
        m["xt6"] = np.ascontiguousarray(x[c].T).astype(bf16)
        in_maps.append(m)
    return in_maps


def kernel(**inputs) -> np.ndarray:
    from concourse.bass_utils import run_bass_kernel_spmd

    nc = _build_graph()
    in_maps = _prep_inputs(**inputs)
    res = run_bass_kernel_spmd(nc, in_maps, core_ids=list(range(NCORES)))
    outs = [np.asarray(res.results[c]["out"], np.float32) for c in range(NCORES)]
    return np.stack(outs)  # [8, 1, 768]
